# revision 44
# baseline (speedup 1.0000x reference)
"""MoE routing kernel for Trainium2 (8 NeuronCores, paired-expert F-sharding).

Sharding strategy:
  - The host computes the gate (same math as the reference, on CPU jax so
    tie-breaking matches bit-for-bit) and pairs experts heavy-with-light
    by routed load (e.g. 592+453). Core pair (2g, 2g+1) owns expert pair
    g: each core of the pair processes ALL of both experts' routed tokens
    but only HALF of the F channels (F-sharding), so per-core compute is
    uniform across the chip regardless of routing imbalance.
  - The shared expert is token-sharded across the 4 groups (512 tokens
    each) and F-sharded across the pair, same emitter.
  - Every core therefore runs 3 FFN batches: expert-A tokens (cap 656),
    expert-B tokens (cap 528), shared slice (512) — 1696 columns of
    half-F work ≈ the ideal 64.4 GFLOP / 8 cores.
  - All matmul operands are bf16 (fp32 PSUM accumulate): halves HBM
    traffic; end-to-end quantization error ~5e-3 absmax-rel.
  - Unshard on host: partial outputs of the two F-halves (bf16) add;
    shared slices concatenate; routed outputs scatter-add by token index.
    Combine weights are applied on-device (ACT scale); the down-proj
    biases (b2*cw, sb2) are added on the host during combination.

A dense all-on-device fallback (every core processes all tokens through
its expert, masked by gate weights computed on-device) is kept for the
(never observed) case that a pair's load exceeds capacity.
"""

import numpy as np
from contextlib import ExitStack

import ml_dtypes

import concourse.bass as bass
import concourse.mybir as mybir
import concourse.tile as tile
from concourse import bacc
from concourse.bass_utils import run_bass_kernel_spmd

# Problem dims (hardcoded per contract)
E = 8
D = 1024
F = 1024
T = 2048          # B*S = 2*1024
P = 128
DK = D // P       # 8 k-chunks over D
FH = F // 2       # 512 channels per core (F-shard half)
FI = FH // P      # 4 f-chunks per branch per core
ALPHA = 1.702
LIMIT = 7.0
NCORES = 8
NGROUPS = 4
CAP_A = 656       # cap for the heavier expert of each pair (max load 642)
CAP_B = 512       # cap for the lighter expert (max load 508; 4 full tiles)
CAP_S = T // NGROUPS  # shared-expert tokens per group

F32 = mybir.dt.float32
F32R = mybir.dt.float32r
BF16 = mybir.dt.bfloat16
AF = mybir.ActivationFunctionType
OP = mybir.AluOpType

BF = ml_dtypes.bfloat16


def _chunks(n):
    # near-equal chunks <= 512 (a tiny tail chunk wastes the ~60-cycle
    # matmul issue floor, so balance instead: 528 -> 264+264)
    k = -(-n // 512)
    base = n // k
    rem = n - base * k
    out = []
    o = 0
    for i in range(k):
        s = base + (1 if i < rem else 0)
        out.append((o, s))
        o += s
    return out


# ---------------------------------------------------------------------------
# generic FFN-batch emitter (half-F):
#   out[cap, D] = cw * (swiglu((xT@w1+b1)*(xT@w3+b3)) @ (w2T/alpha) + b2)
# where swiglu' returns alpha*a (the 1/alpha is folded into w2T on host).
# ---------------------------------------------------------------------------

def _emit_loads(tc, pools, pref, aps, cap, first=False):
    """Issue the front-of-batch DMAs. The per-DMA issue cost on a HWDGE
    queue is ~0.6us, so the x tiles are split even/odd across the sync
    and scalar queues (x tiles are fresh buffers and never carry waits,
    so they cannot stall ACT compute queued behind them). For the first
    batch the fi=0 w3 weights ride the scalar queue too, halving the
    issue latency in front of the opening matmul groups."""
    nc = tc.nc
    wA, w2p, apool, hpool, outp, psA, psB = pools

    wt0 = wA.tile([P, 4, DK, P], BF16, tag="wq")
    nc.sync.dma_start(wt0[:], aps[pref + "wq"][0])

    xsb = []
    for dk in range(DK):
        t = apool.tile([P, cap], BF16, tag=f"{pref}x{dk}")
        eng = nc.sync if dk % 2 == 0 else nc.scalar
        eng.dma_start(t[:], aps[pref + "xT"][dk * P:(dk + 1) * P, :])
        xsb.append(t)

    ball = apool.tile([P, 4 * FI], F32, tag=pref + "ball")
    nc.sync.dma_start(ball[:], aps[pref + "ball"][:])
    return dict(wt0=wt0, xsb=xsb, ball=ball, first=first)


def _emit_stage_a(tc, pools, pref, aps, cap, tiles):
    nc = tc.nc
    wA, w2p, apool, hpool, outp, psA, psB = pools
    wt0, xsb, ball = tiles["wt0"], tiles["xsb"], tiles["ball"]
    ntt = (cap + P - 1) // P

    atiles = []
    for fi in range(FI):
        at = apool.tile([P, cap], BF16, tag=f"{pref}a{fi}")
        atiles.append(at)
        if fi == 0:
            wt = wt0
        else:
            wt = wA.tile([P, 4, DK, P], BF16, tag="wq")
            nc.sync.dma_start(wt[:], aps[pref + "wq"][fi])
        if fi == FI - 1:
            # queue this batch's stage-B loads behind the last weight DMA
            cwt = apool.tile([P, ntt], F32, tag=pref + "cw")
            nc.sync.dma_start(cwt[:], aps[pref + "cw"][:])
            w2t = w2p.tile([P, FI, D], BF16, tag=pref + "w2q")
            nc.sync.dma_start(w2t[:], aps[pref + "w2q"][:])
            tiles["cwt"], tiles["w2t"] = cwt, w2t
        bc1g = ball[:, 0 * FI + fi:0 * FI + fi + 1]
        bc1l = ball[:, 1 * FI + fi:1 * FI + fi + 1]
        bc3g = ball[:, 2 * FI + fi:2 * FI + fi + 1]
        bc3l = ball[:, 3 * FI + fi:3 * FI + fi + 1]

        for (to, ts) in _chunks(cap):
            tsl = slice(to, to + ts)

            def hpsum(wj, ptag):
                ps = psA.tile([P, 512], F32, tag=ptag)
                for dk in range(DK):
                    nc.tensor.matmul(
                        ps[:, :ts], (wt[:, wj, dk, :]),
                        (xsb[dk][:, tsl]),
                        start=(dk == 0), stop=(dk == DK - 1))
                return ps

            pg1 = hpsum(0, "pA")
            t1 = hpool.tile([P, 512], F32, tag="tcp")
            nc.scalar.activation(t1[:, :ts], pg1[:, :ts], AF.Identity,
                                 bias=bc1g)
            pg3 = hpsum(2, "pB")
            hg = hpool.tile([P, 512], F32, tag="hh")
            nc.vector.scalar_tensor_tensor(
                out=hg[:, :ts], in0=pg3[:, :ts], scalar=bc3g, in1=t1[:, :ts],
                op0=OP.add, op1=OP.mult)
            nc.vector.tensor_scalar_min(hg[:, :ts], hg[:, :ts], LIMIT)
            gs = hpool.tile([P, 512], F32, tag="gs")
            nc.scalar.activation(gs[:, :ts], hg[:, :ts], AF.Silu, scale=ALPHA)

            pl1 = hpsum(1, "pA")
            t2 = hpool.tile([P, 512], F32, tag="tcp")
            nc.scalar.activation(t2[:, :ts], pl1[:, :ts], AF.Identity,
                                 bias=bc1l)
            pl3 = hpsum(3, "pB")
            hl = hpool.tile([P, 512], F32, tag="hh")
            nc.vector.scalar_tensor_tensor(
                out=hl[:, :ts], in0=pl3[:, :ts], scalar=bc3l, in1=t2[:, :ts],
                op0=OP.add, op1=OP.mult)
            nc.vector.tensor_scalar(
                out=hl[:, :ts], in0=hl[:, :ts], scalar1=LIMIT, scalar2=-LIMIT,
                op0=OP.min, op1=OP.max)
            # a = (hl + 1) * gs   (the 1/alpha lives in w2T)
            nc.vector.scalar_tensor_tensor(
                out=atiles[fi][:, tsl], in0=hl[:, :ts], scalar=1.0,
                in1=gs[:, :ts], op0=OP.add, op1=OP.mult)
    tiles["atiles"] = atiles


def _emit_stage_b(tc, pools, pref, aps, cap, tiles, last=False):
    # b2 is added on the host; DVE applies the combine weight and writes
    # bf16 partials; the next batch's loads were already queued before
    # these stores, so sync-queue ordering cannot starve the PE. On the
    # final batch the trailing groups drain on ACT as well so the
    # end-of-program backlog clears twice as fast.
    nc = tc.nc
    wA, w2p, apool, hpool, outp, psA, psB = pools
    atiles, cwt, w2t = tiles["atiles"], tiles["cwt"], tiles["w2t"]
    ntt = (cap + P - 1) // P
    ngrp = ntt * (D // 512)
    g = 0
    for tp in range(ntt):
        tn = min(P, cap - tp * P)
        tsl = slice(tp * P, tp * P + tn)
        ot = outp.tile([P, D], BF16, tag="ot")
        for dch in range(D // 512):
            dsl = slice(dch * 512, (dch + 1) * 512)
            pB = psB.tile([P, 512], F32, tag="pB2")
            for fi in range(FI):
                nc.tensor.matmul(
                    pB[:tn, :], (atiles[fi][:, tsl]), (w2t[:, fi, dsl]),
                    start=(fi == 0), stop=(fi == FI - 1))
            if last and g >= ngrp - 4 and g % 2 == 0:
                nc.scalar.activation(ot[:tn, dsl], pB[:tn, :], AF.Identity,
                                     scale=cwt[:tn, tp:tp + 1])
            else:
                nc.vector.tensor_scalar_mul(
                    ot[:tn, dsl], pB[:tn, :], cwt[:tn, tp:tp + 1])
            g += 1
        nc.sync.dma_start(aps[pref + "out"][tsl, :], ot[:tn, :])


def _build_sparse():
    nc = bacc.Bacc(
        "TRN2", target_bir_lowering=False, debug=False, num_devices=NCORES
    )
    aps = {}

    def inp(name, shape, dt=F32):
        aps[name] = nc.dram_tensor(name, shape, dt, kind="ExternalInput").ap()

    for pref, cap in (("a_", CAP_A), ("b_", CAP_B), ("s_", CAP_S)):
        inp(pref + "xT", [D, cap], BF16)
        inp(pref + "wq", [FI, P, 4, DK, P], BF16)
        inp(pref + "ball", [P, 4 * FI])
        inp(pref + "w2q", [P, FI, D], BF16)
        inp(pref + "cw", [P, (cap + P - 1) // P])
        aps[pref + "out"] = nc.dram_tensor(
            pref + "out", [cap, D], BF16, kind="ExternalOutput").ap()

    with tile.TileContext(nc) as tc:
        with ExitStack() as ctx:
            wA = ctx.enter_context(tc.tile_pool(name="wA", bufs=4))
            w2p = ctx.enter_context(tc.tile_pool(name="w2p", bufs=1))
            apool = ctx.enter_context(tc.tile_pool(name="apool", bufs=1))
            hpool = ctx.enter_context(tc.tile_pool(name="hpool", bufs=3))
            outp = ctx.enter_context(tc.tile_pool(name="outp", bufs=4))
            psA = ctx.enter_context(
                tc.tile_pool(name="psA", bufs=2, space="PSUM"))
            psB = ctx.enter_context(
                tc.tile_pool(name="psB", bufs=4, space="PSUM"))
            pools = (wA, w2p, apool, hpool, outp, psA, psB)
            batches = (("a_", CAP_A), ("b_", CAP_B), ("s_", CAP_S))
            tiles = {}
            for i, (pref, cap) in enumerate(batches):
                if i == 0:
                    tiles[pref] = _emit_loads(
                        tc, pools, pref, aps, cap, first=True)
                _emit_stage_a(tc, pools, pref, aps, cap, tiles[pref])
                if i + 1 < len(batches):
                    nx, ncap = batches[i + 1]
                    tiles[nx] = _emit_loads(tc, pools, nx, aps, ncap)
                _emit_stage_b(tc, pools, pref, aps, cap, tiles[pref],
                              last=(i == len(batches) - 1))
    nc.compile()
    return nc


# ---------------------------------------------------------------------------
# host-side prep
# ---------------------------------------------------------------------------

def _warr(w):      # [FH, D] -> [FI, P, DK, P] stage-A stationary layout
    return np.ascontiguousarray(
        w.T.reshape(DK, P, FI, P).transpose(2, 1, 0, 3))


def _bcol(b):      # [FH] -> [P, FI]
    return np.ascontiguousarray(b.reshape(FI, P).T)


def _gate(x, gate_w, gate_b):
    """Replicate the reference gate on CPU jax (bit-identical math)."""
    import jax
    import jax.numpy as jnp
    cpu = jax.devices("cpu")[0]
    with jax.default_device(cpu):
        xt = jnp.asarray(np.asarray(x, np.float32).reshape(T, D))
        logits = xt @ jnp.asarray(np.asarray(gate_w, np.float32)).T
        scores = jax.nn.softmax(logits.astype(jnp.float32), axis=-1)
        biased = scores + jnp.asarray(
            np.asarray(gate_b, np.float32)).astype(jnp.float32)
        idx = jax.lax.top_k(biased, 2)[1]
        weights = jnp.take_along_axis(scores, idx, axis=-1)
        return np.asarray(idx), np.asarray(weights)


def _prep_sparse(x, gate_w, gate_b, w1, b1, w3, b3, w2, b2,
                 sw1, sb1, sw3, sb3, sw2, sb2):
    f32 = np.float32
    xt = np.asarray(x, f32).reshape(T, D)
    xTq = np.ascontiguousarray(xt.T.astype(BF))     # [D, T] bf16

    idx, wts = _gate(x, gate_w, gate_b)             # [T, 2], [T, 2]
    toks = [[] for _ in range(E)]
    cws = [[] for _ in range(E)]
    for k in range(2):
        for t in range(T):
            e = int(idx[t, k])
            toks[e].append(t)
            cws[e].append(wts[t, k])
    counts = np.array([len(v) for v in toks])

    # pair heavy-with-light
    order = np.argsort(counts, kind='stable')
    eAs = [int(order[7 - g]) for g in range(NGROUPS)]   # heavier experts
    eBs = [int(order[g]) for g in range(NGROUPS)]       # lighter experts
    if counts[eAs].max() > CAP_A or counts[eBs].max() > CAP_B:
        return None, None, None  # fall back to dense

    def halves(w1e, b1e, w3e, b3e, w2e, b2e):
        """Per-F-half weight dict pieces for one expert's matrices."""
        w1e, w3e = np.asarray(w1e, f32), np.asarray(w3e, f32)
        b1e, b3e = np.asarray(b1e, f32), np.asarray(b3e, f32)
        w2e, b2e = np.asarray(w2e, f32), np.asarray(b2e, f32)
        out = []
        for h in range(2):
            fsl = slice(h * FH, (h + 1) * FH)
            wq = np.stack([_warr(w1e[0::2][fsl].astype(BF)),
                           _warr(w1e[1::2][fsl].astype(BF)),
                           _warr(w3e[0::2][fsl].astype(BF)),
                           _warr(w3e[1::2][fsl].astype(BF))], axis=2)
            w2q = (w2e.T[fsl] * (1.0 / ALPHA)).astype(BF)
            out.append({
                "wq": np.ascontiguousarray(wq),
                "ball": np.ascontiguousarray(np.concatenate(
                    [_bcol(b1e[0::2][fsl]), _bcol(b1e[1::2][fsl]),
                     _bcol(b3e[0::2][fsl]), _bcol(b3e[1::2][fsl])], axis=1)),
                "w2q": np.ascontiguousarray(
                    w2q.reshape(FI, P, D).transpose(1, 0, 2)),
            })
        return out

    def gather(tl, cwv, cap):
        n = len(tl)
        tpad = np.zeros(cap, np.int64)
        tpad[:n] = tl
        cpad = np.zeros(((cap + P - 1) // P) * P, f32)
        cpad[:n] = cwv
        xg = np.ascontiguousarray(xTq[:, tpad])
        cwcol = np.ascontiguousarray(
            cpad.reshape(-1, P).T)
        return xg, cwcol

    sh_halves = halves(sw1, sb1, sw3, sb3, sw2, sb2)

    in_maps = [dict() for _ in range(NCORES)]
    meta = []
    for g in range(NGROUPS):
        eA, eB = eAs[g], eBs[g]
        xgA, cwA = gather(toks[eA], cws[eA], CAP_A)
        xgB, cwB = gather(toks[eB], cws[eB], CAP_B)
        ssl = slice(g * CAP_S, (g + 1) * CAP_S)
        xs = np.ascontiguousarray(xTq[:, ssl])
        scw = np.ones((P, CAP_S // P), f32)
        meta.append((toks[eA], counts[eA], np.asarray(cws[eA], f32), eA,
                     toks[eB], counts[eB], np.asarray(cws[eB], f32), eB))
        hvA = halves(w1[eA], b1[eA], w3[eA], b3[eA], w2[eA], b2[eA])
        hvB = halves(w1[eB], b1[eB], w3[eB], b3[eB], w2[eB], b2[eB])
        for h in range(2):
            c = 2 * g + h
            m = in_maps[c]
            for pref, hv in (("a_", hvA[h]), ("b_", hvB[h])):
                for k, v in hv.items():
                    m[pref + k] = v
            for k, v in sh_halves[h].items():
                m["s_" + k] = v
            m["a_xT"], m["a_cw"] = xgA, cwA
            m["b_xT"], m["b_cw"] = xgB, cwB
            m["s_xT"], m["s_cw"] = xs, scw
    return in_maps, meta, None


_PROGS = {}


def _get_program(kind):
    if kind not in _PROGS:
        _PROGS[kind] = {"sparse": _build_sparse, "dense": _build_dense}[kind]()
    return _PROGS[kind]


def kernel(x, gate_w, gate_b, w1, b1, w3, b3, w2, b2,
           sw1, sb1, sw3, sb3, sw2, sb2, _trace=False, _results=None,
           _force_dense=False):
    kw = {}
    if _trace:
        kw = dict(trace=True, trace_cores=list(range(NCORES)))
    args = (x, gate_w, gate_b, w1, b1, w3, b3, w2, b2,
            sw1, sb1, sw3, sb3, sw2, sb2)
    if not _force_dense:
        in_maps, meta, _ = _prep_sparse(*args)
    else:
        in_maps = None
    if in_maps is not None:
        nc = _get_program("sparse")
        res = run_bass_kernel_spmd(
            nc, in_maps, core_ids=list(range(NCORES)), **kw)
        if _results is not None:
            _results.append(res)
        f32 = np.float32
        out = np.zeros((T, D), f32)
        for g in range(NGROUPS):
            r0, r1 = res.results[2 * g], res.results[2 * g + 1]
            out[g * CAP_S:(g + 1) * CAP_S] = (
                r0["s_out"].astype(f32) + r1["s_out"].astype(f32))
        out += np.asarray(sb2, f32)          # shared-expert down bias
        for g in range(NGROUPS):
            r0, r1 = res.results[2 * g], res.results[2 * g + 1]
            tA, nA, cwA, eA, tB, nB, cwB, eB = meta[g]
            out[tA] += (r0["a_out"][:nA].astype(f32)
                        + r1["a_out"][:nA].astype(f32)
                        + cwA[:nA, None] * np.asarray(b2[eA], f32))
            out[tB] += (r0["b_out"][:nB].astype(f32)
                        + r1["b_out"][:nB].astype(f32)
                        + cwB[:nB, None] * np.asarray(b2[eB], f32))
        return out.reshape(np.asarray(x).shape).astype(np.float32)

    # dense fallback
    in_maps = _prep_dense(*args)
    nc = _get_program("dense")
    res = run_bass_kernel_spmd(nc, in_maps, core_ids=list(range(NCORES)), **kw)
    if _results is not None:
        _results.append(res)
    acc = np.zeros((T, D), np.float32)
    for c in range(NCORES):
        acc += res.results[c]["out"]
    return acc.reshape(np.asarray(x).shape).astype(np.float32)


# ---------------------------------------------------------------------------
# dense all-on-device fallback (V1): every core runs its expert over all
# tokens, masked by on-device gate weights; shared expert sharded on 2F.
# ---------------------------------------------------------------------------

TCH = 512
NTH = 2
TH = T // NTH
DKF = D // P
FIF = F // P


def _build_dense():
    nc = bacc.Bacc(
        "TRN2", target_bir_lowering=False, debug=False, num_devices=NCORES
    )
    aps = {}

    def inp(name, shape, dt=F32):
        aps[name] = nc.dram_tensor(name, shape, dt, kind="ExternalInput").ap()

    inp("xT", [D, T], F32R)
    inp("gw", [P, DKF * E], F32R)
    inp("gb", [P, E])
    inp("sel", [P, E])
    for n in ("w1g", "w1l", "w3g", "w3l"):
        inp(n, [FIF, P, DKF, P], F32R)
    for n in ("b1g", "b1l", "b3g", "b3l"):
        inp(n, [P, FIF + 1])
    inp("w2T", [F, D], F32R)
    inp("b2r", [1, D], F32R)
    for n in ("sw1g", "sw1l", "sw3g", "sw3l"):
        inp(n, [P, DKF, P], F32R)
    inp("sw2T", [P, D], F32R)
    inp("sb2r", [1, D], F32R)
    inp("ones", [1, P], F32R)
    aps["out"] = nc.dram_tensor("out", [T, D], F32, kind="ExternalOutput").ap()

    with tile.TileContext(nc) as tc:
        _emit_dense(tc, aps)
    nc.compile()
    return nc


def _emit_dense(tc, aps):
    nc = tc.nc
    ctx = ExitStack()

    with ctx:
        const = ctx.enter_context(tc.tile_pool(name="const", bufs=1))

        xsb = []
        for dk in range(DKF):
            t = const.tile([P, T], F32R, tag=f"x{dk}")
            nc.sync.dma_start(t[:], aps["xT"][dk * P:(dk + 1) * P, :])
            xsb.append(t)

        def load_const(name, shape, dt=F32):
            t = const.tile(shape, dt, tag=name)
            nc.sync.dma_start(t[:], aps[name][:])
            return t

        gw_sb = load_const("gw", [P, DKF * E], F32R)
        gb_sb = load_const("gb", [P, E])
        sel_sb = load_const("sel", [P, E])
        bcols = {n: load_const(n, [P, FIF + 1])
                 for n in ("b1g", "b1l", "b3g", "b3l")}
        b2r_sb = load_const("b2r", [1, D], F32R)
        sb2r_sb = load_const("sb2r", [1, D], F32R)
        sw2T_sb = load_const("sw2T", [P, D], F32R)
        ssw = {}
        for name in ("sw1g", "sw1l", "sw3g", "sw3l"):
            t = const.tile([P, DKF, P], F32R, tag=name)
            nc.sync.dma_start(t[:], aps[name][:])
            ssw[name] = t

        ones = const.tile([1, P], F32R, tag="ones")
        nc.sync.dma_start(ones[:], aps["ones"][:])
        ident = const.tile([E, E], F32, tag="ident")
        nc.vector.memset(ident[:], 0.0)
        from concourse.masks import make_identity
        make_identity(nc, ident[:], nomemset=True)

        cw = const.tile([P, T // P], F32, tag="cw")

        # ---- gate ----
        with tc.tile_pool(name="psG", bufs=2, space="PSUM") as psG, \
             tc.tile_pool(name="gtmp", bufs=1) as gtmp:
            NC = T // P
            logits_tb = const.tile([P, NC * E], F32, tag="logits_tb")
            logitsT = gtmp.tile([E, T], F32, tag="logitsT")
            for tch in range(T // TCH):
                pg = psG.tile([E, TCH], F32, tag="pslog")
                for dk in range(DKF):
                    nc.tensor.matmul(
                        pg[:],
                        (gw_sb[:, dk * E:(dk + 1) * E]),
                        (xsb[dk][:, tch * TCH:(tch + 1) * TCH]),
                        start=(dk == 0), stop=(dk == DKF - 1),
                    )
                nc.scalar.copy(logitsT[:, tch * TCH:(tch + 1) * TCH], pg[:])
            for j in range(NC):
                pt = psG.tile([P, E], F32, tag="pstr")
                nc.tensor.transpose(
                    pt[:], logitsT[:, j * P:(j + 1) * P], ident[:])
                nc.scalar.copy(logits_tb[:, j * E:(j + 1) * E], pt[:])

            eL = gtmp.tile([P, NC * E], F32, tag="eL")
            nc.scalar.activation(eL[:], logits_tb[:], AF.Exp)
            e3 = eL[:].rearrange("p (c e) -> p c e", e=E)
            ssum = gtmp.tile([P, NC], F32, tag="ssum")
            nc.vector.reduce_sum(ssum[:], e3, axis=mybir.AxisListType.X)
            rs = gtmp.tile([P, NC], F32, tag="rs")
            nc.vector.reciprocal(rs[:], ssum[:])
            scores = gtmp.tile([P, NC * E], F32, tag="scores")
            s3 = scores[:].rearrange("p (c e) -> p c e", e=E)
            nc.vector.tensor_mul(
                s3, e3, rs[:, :, None].broadcast_to((P, NC, E)))
            biased = gtmp.tile([P, NC * E], F32, tag="biased")
            bi3 = biased[:].rearrange("p (c e) -> p c e", e=E)
            nc.vector.tensor_add(
                bi3, s3, gb_sb[:, None, :].broadcast_to((P, NC, E)))
            m1 = gtmp.tile([P, NC], F32, tag="m1")
            nc.vector.reduce_max(m1[:], bi3, axis=mybir.AxisListType.X)
            mask1 = gtmp.tile([P, NC * E], F32, tag="mask1")
            mk3 = mask1[:].rearrange("p (c e) -> p c e", e=E)
            nc.vector.tensor_tensor(
                mk3, bi3, m1[:, :, None].broadcast_to((P, NC, E)), OP.is_ge)
            biased2 = gtmp.tile([P, NC * E], F32, tag="biased2")
            b23 = biased2[:].rearrange("p (c e) -> p c e", e=E)
            nc.vector.scalar_tensor_tensor(
                out=b23, in0=mk3, scalar=-1e30, in1=bi3,
                op0=OP.mult, op1=OP.add)
            m2 = gtmp.tile([P, NC], F32, tag="m2")
            nc.vector.reduce_max(m2[:], b23, axis=mybir.AxisListType.X)
            mask2 = gtmp.tile([P, NC * E], F32, tag="mask2")
            mq3 = mask2[:].rearrange("p (c e) -> p c e", e=E)
            nc.vector.tensor_tensor(
                mq3, bi3, m2[:, :, None].broadcast_to((P, NC, E)), OP.is_ge)
            cwf = gtmp.tile([P, NC * E], F32, tag="cwf")
            cf3 = cwf[:].rearrange("p (c e) -> p c e", e=E)
            nc.vector.tensor_mul(cf3, s3, mq3)
            nc.vector.tensor_mul(
                cf3, cf3, sel_sb[:, None, :].broadcast_to((P, NC, E)))
            nc.vector.reduce_sum(cw[:], cf3, axis=mybir.AxisListType.X)

        # ---- main ----
        wA = ctx.enter_context(tc.tile_pool(name="wA", bufs=2))
        w2p = ctx.enter_context(tc.tile_pool(name="w2p", bufs=3))
        apool = ctx.enter_context(tc.tile_pool(name="apool", bufs=1))
        hpool = ctx.enter_context(tc.tile_pool(name="hpool", bufs=2))
        outp = ctx.enter_context(tc.tile_pool(name="outp", bufs=3))
        psA = ctx.enter_context(tc.tile_pool(name="psA", bufs=2, space="PSUM"))
        psB = ctx.enter_context(tc.tile_pool(name="psB", bufs=2, space="PSUM"))
        psS = ctx.enter_context(tc.tile_pool(name="psS", bufs=2, space="PSUM"))

        afc = FIF + 1
        for th in range(NTH):
            tbase = th * TH
            atiles = []
            for fi in range(afc):
                at = apool.tile([P, TH], F32R, tag=f"a{fi}")
                atiles.append(at)
                if fi < FIF:
                    wt = {}
                    for nm in ("w1g", "w1l", "w3g", "w3l"):
                        t = wA.tile([P, DKF, P], F32R, tag=nm)
                        nc.sync.dma_start(t[:], aps[nm][fi])
                        wt[nm] = t
                    w_g1, w_l1 = wt["w1g"], wt["w1l"]
                    w_g3, w_l3 = wt["w3g"], wt["w3l"]
                else:
                    w_g1, w_l1 = ssw["sw1g"], ssw["sw1l"]
                    w_g3, w_l3 = ssw["sw3g"], ssw["sw3l"]
                bc1g = bcols["b1g"][:, fi:fi + 1]
                bc1l = bcols["b1l"][:, fi:fi + 1]
                bc3g = bcols["b3g"][:, fi:fi + 1]
                bc3l = bcols["b3l"][:, fi:fi + 1]

                for tt in range(TH // TCH):
                    tsl = slice(tt * TCH, (tt + 1) * TCH)
                    gsl = slice(tbase + tt * TCH, tbase + (tt + 1) * TCH)

                    def hpsum(wtile, ptag):
                        ps = psA.tile([P, TCH], F32, tag=ptag)
                        for dk in range(DKF):
                            nc.tensor.matmul(
                                ps[:], (wtile[:, dk, :]),
                                (xsb[dk][:, gsl]),
                                start=(dk == 0), stop=(dk == DKF - 1))
                        return ps

                    pg1 = hpsum(w_g1, "pA")
                    t1 = hpool.tile([P, TCH], F32, tag="tcp")
                    nc.scalar.activation(t1[:], pg1[:], AF.Identity, bias=bc1g)
                    pg3 = hpsum(w_g3, "pB")
                    hg = hpool.tile([P, TCH], F32, tag="hh")
                    nc.vector.scalar_tensor_tensor(
                        out=hg[:], in0=pg3[:], scalar=bc3g, in1=t1[:],
                        op0=OP.add, op1=OP.mult)
                    nc.vector.tensor_scalar_min(hg[:], hg[:], LIMIT)
                    gs = hpool.tile([P, TCH], F32, tag="gs")
                    nc.scalar.activation(gs[:], hg[:], AF.Silu, scale=ALPHA)

                    pl1 = hpsum(w_l1, "pA")
                    t2 = hpool.tile([P, TCH], F32, tag="tcp")
                    nc.scalar.activation(t2[:], pl1[:], AF.Identity, bias=bc1l)
                    pl3 = hpsum(w_l3, "pB")
                    hl = hpool.tile([P, TCH], F32, tag="hh")
                    nc.vector.scalar_tensor_tensor(
                        out=hl[:], in0=pl3[:], scalar=bc3l, in1=t2[:],
                        op0=OP.add, op1=OP.mult)
                    nc.vector.tensor_scalar(
                        out=hl[:], in0=hl[:], scalar1=LIMIT, scalar2=-LIMIT,
                        op0=OP.min, op1=OP.max)
                    nc.vector.tensor_scalar(
                        out=hl[:], in0=hl[:], scalar1=1.0 / ALPHA,
                        scalar2=1.0 / ALPHA, op0=OP.mult, op1=OP.add)
                    nc.vector.tensor_mul(atiles[fi][:, tsl], gs[:], hl[:])

            for tp in range(TH // P):
                j = th * (TH // P) + tp
                tsl = slice(tp * P, (tp + 1) * P)
                for dch in range(D // TCH):
                    dsl = slice(dch * TCH, (dch + 1) * TCH)
                    pB = psB.tile([P, TCH], F32, tag="pB2")
                    nc.tensor.matmul(pB[:], (ones[:]),
                                     (b2r_sb[0:1, dsl]),
                                     start=True, stop=False)
                    for fi in range(FIF):
                        wt2 = w2p.tile([P, TCH], F32R, tag="w2t")
                        nc.sync.dma_start(
                            wt2[:], aps["w2T"][fi * P:(fi + 1) * P, dsl])
                        nc.tensor.matmul(
                            pB[:], (atiles[fi][:, tsl]), (wt2[:]),
                            start=False, stop=(fi == FIF - 1))
                    pS = psS.tile([P, TCH], F32, tag="pS")
                    nc.tensor.matmul(pS[:], (ones[:]),
                                     (sb2r_sb[0:1, dsl]),
                                     start=True, stop=False)
                    nc.tensor.matmul(
                        pS[:], (atiles[FIF][:, tsl]), (sw2T_sb[:, dsl]),
                        start=False, stop=True)
                    ot = outp.tile([P, TCH], F32, tag="ot")
                    nc.vector.tensor_scalar_mul(ot[:], pB[:], cw[:, j:j + 1])
                    nc.vector.tensor_add(ot[:], pS[:], ot[:])
                    nc.sync.dma_start(
                        aps["out"][tbase + tp * P:tbase + (tp + 1) * P, dsl],
                        ot[:])


def _prep_dense(x, gate_w, gate_b, w1, b1, w3, b3, w2, b2,
                sw1, sb1, sw3, sb3, sw2, sb2):
    f32 = np.float32
    xt = np.asarray(x, f32).reshape(T, D)
    xT = np.ascontiguousarray(xt.T)
    gwT = np.asarray(gate_w, f32).T
    gw_sb = np.ascontiguousarray(
        gwT.reshape(DKF, P, E).transpose(1, 0, 2).reshape(P, DKF * E))
    gb_bc = np.ascontiguousarray(
        np.broadcast_to(np.asarray(gate_b, f32), (P, E)))

    sw1 = np.asarray(sw1, f32)
    sw3 = np.asarray(sw3, f32)
    sw2T = np.asarray(sw2, f32).T
    sb1 = np.asarray(sb1, f32)
    sb3 = np.asarray(sb3, f32)
    sb2 = np.asarray(sb2, f32)

    def fwarr(w):      # [F, D] -> [FIF, P, DKF, P]
        return np.ascontiguousarray(
            w.T.reshape(DKF, P, FIF, P).transpose(2, 1, 0, 3))

    def swarr(w_sl):
        return np.ascontiguousarray(
            w_sl.T.reshape(DKF, P, P).transpose(1, 0, 2))

    def bcol2(b, sb_sl):
        return np.ascontiguousarray(
            np.concatenate([b.reshape(FIF, P).T, sb_sl[:, None]], axis=1))

    in_maps = []
    for c in range(NCORES):
        sel = np.zeros((P, E), f32)
        sel[:, c] = 1.0
        w1c = np.asarray(w1[c], f32)
        w3c = np.asarray(w3[c], f32)
        b1c = np.asarray(b1[c], f32)
        b3c = np.asarray(b3[c], f32)
        fsl = slice(c * P, (c + 1) * P)
        m = {
            "xT": xT, "gw": gw_sb, "gb": gb_bc, "sel": sel,
            "w1g": fwarr(w1c[0::2]), "w1l": fwarr(w1c[1::2]),
            "w3g": fwarr(w3c[0::2]), "w3l": fwarr(w3c[1::2]),
            "b1g": bcol2(b1c[0::2], sb1[0::2][fsl]),
            "b1l": bcol2(b1c[1::2], sb1[1::2][fsl]),
            "b3g": bcol2(b3c[0::2], sb3[0::2][fsl]),
            "b3l": bcol2(b3c[1::2], sb3[1::2][fsl]),
            "w2T": np.ascontiguousarray(np.asarray(w2[c], f32).T),
            "b2r": np.asarray(b2[c], f32)[None, :],
            "sw1g": swarr(sw1[0::2][fsl]), "sw1l": swarr(sw1[1::2][fsl]),
            "sw3g": swarr(sw3[0::2][fsl]), "sw3l": swarr(sw3[1::2][fsl]),
            "sw2T": np.ascontiguousarray(sw2T[fsl]),
            "sb2r": (sb2 if c == 0 else np.zeros_like(sb2))[None, :],
            "ones": np.ones((1, P), f32),
        }
        in_maps.append(m)
    return in_maps


if __name__ == "__main__":
    rng = np.random.RandomState(0)
    sd = 1 / 32.0
    ins = {
        "x": rng.randn(2, 1024, 1024).astype(np.float32),
        "gate_w": (rng.randn(E, D) * sd).astype(np.float32),
        "gate_b": (rng.randn(E) * 0.01).astype(np.float32),
        "w1": (rng.randn(E, 2 * F, D) * sd).astype(np.float32),
        "b1": (rng.randn(E, 2 * F) * 0.01).astype(np.float32),
        "w3": (rng.randn(E, 2 * F, D) * sd).astype(np.float32),
        "b3": (rng.randn(E, 2 * F) * 0.01).astype(np.float32),
        "w2": (rng.randn(E, D, F) * sd).astype(np.float32),
        "b2": (rng.randn(E, D) * 0.01).astype(np.float32),
        "sw1": (rng.randn(2 * F, D) * sd).astype(np.float32),
        "sb1": (rng.randn(2 * F) * 0.01).astype(np.float32),
        "sw3": (rng.randn(2 * F, D) * sd).astype(np.float32),
        "sb3": (rng.randn(2 * F) * 0.01).astype(np.float32),
        "sw2": (rng.randn(D, F) * sd).astype(np.float32),
        "sb2": (rng.randn(D) * 0.01).astype(np.float32),
    }
    out = kernel(**ins)
    print("OK", out.shape, out.dtype, np.abs(out).mean())


# revision 51
# speedup vs baseline: 1.2193x; 1.2193x over previous
"""MoE routing kernel for Trainium2 (8 NeuronCores, paired-expert F-sharding).

Sharding strategy:
  - The host computes the gate (same math as the reference, on CPU jax so
    tie-breaking matches bit-for-bit) and pairs experts heavy-with-light
    by routed load (e.g. 592+453). Core pair (2g, 2g+1) owns expert pair
    g: each core of the pair processes ALL of both experts' routed tokens
    but only HALF of the F channels (F-sharding), so per-core compute is
    uniform across the chip regardless of routing imbalance.
  - The shared expert is token-sharded across the 4 groups (512 tokens
    each) and F-sharded across the pair, same emitter.
  - Every core therefore runs 3 FFN batches: expert-A tokens (cap 656),
    expert-B tokens (cap 528), shared slice (512) — 1696 columns of
    half-F work ≈ the ideal 64.4 GFLOP / 8 cores.
  - All matmul operands are bf16 (fp32 PSUM accumulate): halves HBM
    traffic; end-to-end quantization error ~5e-3 absmax-rel.
  - Unshard on host: partial outputs of the two F-halves (bf16) add;
    shared slices concatenate; routed outputs scatter-add by token index.
    Combine weights are applied on-device (ACT scale); the down-proj
    biases (b2*cw, sb2) are added on the host during combination.

A dense all-on-device fallback (every core processes all tokens through
its expert, masked by gate weights computed on-device) is kept for the
(never observed) case that a pair's load exceeds capacity.
"""

import numpy as np
from contextlib import ExitStack

import ml_dtypes

import concourse.bass as bass
import concourse.mybir as mybir
import concourse.tile as tile
from concourse import bacc
from concourse.bass_utils import run_bass_kernel_spmd

# Problem dims (hardcoded per contract)
E = 8
D = 1024
F = 1024
T = 2048          # B*S = 2*1024
P = 128
DK = D // P       # 8 k-chunks over D
FH = F // 2       # 512 channels per core (F-shard half)
FI = FH // P      # 4 f-chunks per branch per core
ALPHA = 1.702
LIMIT = 7.0
NCORES = 8
NGROUPS = 4
CAP_A = 656       # cap for the heavier expert of each pair (max load 642)
CAP_B = 512       # cap for the lighter expert (max load 508; 4 full tiles)
CAP_S = T // NGROUPS  # shared-expert tokens per group

F32 = mybir.dt.float32
F32R = mybir.dt.float32r
BF16 = mybir.dt.bfloat16
AF = mybir.ActivationFunctionType
OP = mybir.AluOpType

BF = ml_dtypes.bfloat16


def _chunks(n):
    # near-equal chunks <= 512 (a tiny tail chunk wastes the ~60-cycle
    # matmul issue floor, so balance instead: 528 -> 264+264)
    k = -(-n // 512)
    base = n // k
    rem = n - base * k
    out = []
    o = 0
    for i in range(k):
        s = base + (1 if i < rem else 0)
        out.append((o, s))
        o += s
    return out


# ---------------------------------------------------------------------------
# generic FFN-batch emitter (half-F):
#   out[cap, D] = cw * (swiglu((xT@w1+b1)*(xT@w3+b3)) @ (w2T/alpha) + b2)
# where swiglu' returns alpha*a (the 1/alpha is folded into w2T on host).
# ---------------------------------------------------------------------------

def _emit_loads(tc, pools, pref, aps, cap, first=False):
    """Issue the front-of-batch DMAs. The per-DMA issue cost on a HWDGE
    queue is ~0.6us, so the x tiles are split even/odd across the sync
    and scalar queues (x tiles are fresh buffers and never carry waits,
    so they cannot stall ACT compute queued behind them). For the first
    batch the fi=0 w3 weights ride the scalar queue too, halving the
    issue latency in front of the opening matmul groups."""
    nc = tc.nc
    wA, w2p, apool, hpool, outp, psA, psB = pools

    wt0 = {}
    weng = {"w1g": nc.sync, "w1l": nc.sync,
            "w3g": nc.scalar if first else nc.sync,
            "w3l": nc.scalar if first else nc.sync}
    for nm in ("w1g", "w3g"):
        t = wA.tile([P, DK, P], BF16, tag=nm)
        weng[nm].dma_start(t[:], aps[pref + nm][0])
        wt0[nm] = t

    xsb = []
    for dk in range(DK):
        t = apool.tile([P, cap], BF16, tag=f"{pref}x{dk}")
        eng = nc.sync if dk % 2 == 0 else nc.scalar
        eng.dma_start(t[:], aps[pref + "xT"][dk * P:(dk + 1) * P, :])
        xsb.append(t)

    for nm in ("w1l", "w3l"):
        t = wA.tile([P, DK, P], BF16, tag=nm)
        weng[nm].dma_start(t[:], aps[pref + nm][0])
        wt0[nm] = t

    ball = apool.tile([P, 4 * FI], F32, tag=pref + "ball")
    nc.sync.dma_start(ball[:], aps[pref + "ball"][:])
    return dict(wt0=wt0, xsb=xsb, ball=ball, first=first)


def _emit_stage_a(tc, pools, pref, aps, cap, tiles):
    nc = tc.nc
    wA, w2p, apool, hpool, outp, psA, psB = pools
    wt0, xsb, ball = tiles["wt0"], tiles["xsb"], tiles["ball"]
    ntt = (cap + P - 1) // P

    atiles = []
    for fi in range(FI):
        at = apool.tile([P, cap], BF16, tag=f"{pref}a{fi}")
        atiles.append(at)
        if fi == 0:
            wt = wt0
        else:
            wt = {}
            for nm in ("w1g", "w1l", "w3g", "w3l"):
                t = wA.tile([P, DK, P], BF16, tag=nm)
                nc.sync.dma_start(t[:], aps[pref + nm][fi])
                wt[nm] = t
        if fi == FI - 1:
            # queue this batch's stage-B loads behind the last weight DMA
            cwt = apool.tile([P, ntt], F32, tag=pref + "cw")
            nc.sync.dma_start(cwt[:], aps[pref + "cw"][:])
            w2t = []
            for fj in range(FI):
                t = w2p.tile([P, D], BF16, tag=f"{pref}w2t{fj}")
                nc.sync.dma_start(
                    t[:], aps[pref + "w2T"][fj * P:(fj + 1) * P, :])
                w2t.append(t)
            tiles["cwt"], tiles["w2t"] = cwt, w2t
        bc1g = ball[:, 0 * FI + fi:0 * FI + fi + 1]
        bc1l = ball[:, 1 * FI + fi:1 * FI + fi + 1]
        bc3g = ball[:, 2 * FI + fi:2 * FI + fi + 1]
        bc3l = ball[:, 3 * FI + fi:3 * FI + fi + 1]

        for (to, ts) in _chunks(cap):
            tsl = slice(to, to + ts)

            def hpsum(wtile, ptag):
                ps = psA.tile([P, 512], F32, tag=ptag)
                for dk in range(DK):
                    nc.tensor.matmul(
                        ps[:, :ts], (wtile[:, dk, :]),
                        (xsb[dk][:, tsl]),
                        start=(dk == 0), stop=(dk == DK - 1))
                return ps

            pg1 = hpsum(wt["w1g"], "pA")
            t1 = hpool.tile([P, 512], F32, tag="tcp")
            nc.scalar.activation(t1[:, :ts], pg1[:, :ts], AF.Identity,
                                 bias=bc1g)
            pg3 = hpsum(wt["w3g"], "pB")
            hg = hpool.tile([P, 512], F32, tag="hh")
            nc.vector.scalar_tensor_tensor(
                out=hg[:, :ts], in0=pg3[:, :ts], scalar=bc3g, in1=t1[:, :ts],
                op0=OP.add, op1=OP.mult)
            nc.vector.tensor_scalar_min(hg[:, :ts], hg[:, :ts], LIMIT)
            gs = hpool.tile([P, 512], F32, tag="gs")
            nc.scalar.activation(gs[:, :ts], hg[:, :ts], AF.Silu, scale=ALPHA)

            pl1 = hpsum(wt["w1l"], "pA")
            t2 = hpool.tile([P, 512], F32, tag="tcp")
            nc.scalar.activation(t2[:, :ts], pl1[:, :ts], AF.Identity,
                                 bias=bc1l)
            pl3 = hpsum(wt["w3l"], "pB")
            hl = hpool.tile([P, 512], F32, tag="hh")
            nc.vector.scalar_tensor_tensor(
                out=hl[:, :ts], in0=pl3[:, :ts], scalar=bc3l, in1=t2[:, :ts],
                op0=OP.add, op1=OP.mult)
            nc.vector.tensor_scalar(
                out=hl[:, :ts], in0=hl[:, :ts], scalar1=LIMIT, scalar2=-LIMIT,
                op0=OP.min, op1=OP.max)
            # a = (hl + 1) * gs   (the 1/alpha lives in w2T)
            nc.vector.scalar_tensor_tensor(
                out=atiles[fi][:, tsl], in0=hl[:, :ts], scalar=1.0,
                in1=gs[:, :ts], op0=OP.add, op1=OP.mult)
    tiles["atiles"] = atiles


def _emit_stage_b(tc, pools, pref, aps, cap, tiles, last=False):
    # b2 is added on the host; DVE applies the combine weight and writes
    # bf16 partials; the next batch's loads were already queued before
    # these stores, so sync-queue ordering cannot starve the PE. On the
    # final batch the trailing groups drain on ACT as well so the
    # end-of-program backlog clears twice as fast.
    nc = tc.nc
    wA, w2p, apool, hpool, outp, psA, psB = pools
    atiles, cwt, w2t = tiles["atiles"], tiles["cwt"], tiles["w2t"]
    ntt = (cap + P - 1) // P
    ngrp = ntt * (D // 512)
    g = 0
    for tp in range(ntt):
        tn = min(P, cap - tp * P)
        tsl = slice(tp * P, tp * P + tn)
        ot = outp.tile([P, D], BF16, tag="ot")
        for dch in range(D // 512):
            dsl = slice(dch * 512, (dch + 1) * 512)
            pB = psB.tile([P, 512], F32, tag="pB2")
            for fi in range(FI):
                nc.tensor.matmul(
                    pB[:tn, :], (atiles[fi][:, tsl]), (w2t[fi][:, dsl]),
                    start=(fi == 0), stop=(fi == FI - 1))
            if last and g >= ngrp - 4 and g % 2 == 0:
                nc.scalar.activation(ot[:tn, dsl], pB[:tn, :], AF.Identity,
                                     scale=cwt[:tn, tp:tp + 1])
            else:
                nc.vector.tensor_scalar_mul(
                    ot[:tn, dsl], pB[:tn, :], cwt[:tn, tp:tp + 1])
            g += 1
        nc.sync.dma_start(aps[pref + "out"][tsl, :], ot[:tn, :])


def _build_sparse():
    nc = bacc.Bacc(
        "TRN2", target_bir_lowering=False, debug=False, num_devices=NCORES
    )
    aps = {}

    def inp(name, shape, dt=F32):
        aps[name] = nc.dram_tensor(name, shape, dt, kind="ExternalInput").ap()

    for pref, cap in (("a_", CAP_A), ("b_", CAP_B), ("s_", CAP_S)):
        inp(pref + "xT", [D, cap], BF16)
        for n in ("w1g", "w1l", "w3g", "w3l"):
            inp(pref + n, [FI, P, DK, P], BF16)
        inp(pref + "ball", [P, 4 * FI])
        inp(pref + "w2T", [FH, D], BF16)
        inp(pref + "cw", [P, (cap + P - 1) // P])
        aps[pref + "out"] = nc.dram_tensor(
            pref + "out", [cap, D], BF16, kind="ExternalOutput").ap()

    with tile.TileContext(nc) as tc:
        with ExitStack() as ctx:
            wA = ctx.enter_context(tc.tile_pool(name="wA", bufs=4))
            w2p = ctx.enter_context(tc.tile_pool(name="w2p", bufs=1))
            apool = ctx.enter_context(tc.tile_pool(name="apool", bufs=1))
            hpool = ctx.enter_context(tc.tile_pool(name="hpool", bufs=3))
            outp = ctx.enter_context(tc.tile_pool(name="outp", bufs=4))
            psA = ctx.enter_context(
                tc.tile_pool(name="psA", bufs=2, space="PSUM"))
            psB = ctx.enter_context(
                tc.tile_pool(name="psB", bufs=4, space="PSUM"))
            pools = (wA, w2p, apool, hpool, outp, psA, psB)
            batches = (("a_", CAP_A), ("b_", CAP_B), ("s_", CAP_S))
            tiles = {}
            for i, (pref, cap) in enumerate(batches):
                if i == 0:
                    tiles[pref] = _emit_loads(
                        tc, pools, pref, aps, cap, first=True)
                _emit_stage_a(tc, pools, pref, aps, cap, tiles[pref])
                if i + 1 < len(batches):
                    nx, ncap = batches[i + 1]
                    tiles[nx] = _emit_loads(tc, pools, nx, aps, ncap)
                _emit_stage_b(tc, pools, pref, aps, cap, tiles[pref],
                              last=(i == len(batches) - 1))
    nc.compile()
    return nc


# ---------------------------------------------------------------------------
# host-side prep
# ---------------------------------------------------------------------------

def _warr(w):      # [FH, D] -> [FI, P, DK, P] stage-A stationary layout
    return np.ascontiguousarray(
        w.T.reshape(DK, P, FI, P).transpose(2, 1, 0, 3))


def _bcol(b):      # [FH] -> [P, FI]
    return np.ascontiguousarray(b.reshape(FI, P).T)


def _gate(x, gate_w, gate_b):
    """Replicate the reference gate on CPU jax (bit-identical math)."""
    import jax
    import jax.numpy as jnp
    cpu = jax.devices("cpu")[0]
    with jax.default_device(cpu):
        xt = jnp.asarray(np.asarray(x, np.float32).reshape(T, D))
        logits = xt @ jnp.asarray(np.asarray(gate_w, np.float32)).T
        scores = jax.nn.softmax(logits.astype(jnp.float32), axis=-1)
        biased = scores + jnp.asarray(
            np.asarray(gate_b, np.float32)).astype(jnp.float32)
        idx = jax.lax.top_k(biased, 2)[1]
        weights = jnp.take_along_axis(scores, idx, axis=-1)
        return np.asarray(idx), np.asarray(weights)


def _prep_sparse(x, gate_w, gate_b, w1, b1, w3, b3, w2, b2,
                 sw1, sb1, sw3, sb3, sw2, sb2):
    f32 = np.float32
    xt = np.asarray(x, f32).reshape(T, D)
    xTq = np.ascontiguousarray(xt.T.astype(BF))     # [D, T] bf16

    idx, wts = _gate(x, gate_w, gate_b)             # [T, 2], [T, 2]
    toks = [[] for _ in range(E)]
    cws = [[] for _ in range(E)]
    for k in range(2):
        for t in range(T):
            e = int(idx[t, k])
            toks[e].append(t)
            cws[e].append(wts[t, k])
    counts = np.array([len(v) for v in toks])

    # pair heavy-with-light
    order = np.argsort(counts, kind='stable')
    eAs = [int(order[7 - g]) for g in range(NGROUPS)]   # heavier experts
    eBs = [int(order[g]) for g in range(NGROUPS)]       # lighter experts
    if counts[eAs].max() > CAP_A or counts[eBs].max() > CAP_B:
        return None, None, None  # fall back to dense

    def halves(w1e, b1e, w3e, b3e, w2e, b2e):
        """Per-F-half weight dict pieces for one expert's matrices."""
        w1e, w3e = np.asarray(w1e, f32), np.asarray(w3e, f32)
        b1e, b3e = np.asarray(b1e, f32), np.asarray(b3e, f32)
        w2e, b2e = np.asarray(w2e, f32), np.asarray(b2e, f32)
        out = []
        for h in range(2):
            fsl = slice(h * FH, (h + 1) * FH)
            out.append({
                "w1g": _warr(w1e[0::2][fsl].astype(BF)),
                "w1l": _warr(w1e[1::2][fsl].astype(BF)),
                "w3g": _warr(w3e[0::2][fsl].astype(BF)),
                "w3l": _warr(w3e[1::2][fsl].astype(BF)),
                "ball": np.ascontiguousarray(np.concatenate(
                    [_bcol(b1e[0::2][fsl]), _bcol(b1e[1::2][fsl]),
                     _bcol(b3e[0::2][fsl]), _bcol(b3e[1::2][fsl])], axis=1)),
                "w2T": np.ascontiguousarray(
                    (w2e.T[fsl] * (1.0 / ALPHA)).astype(BF)),
            })
        return out

    def gather(tl, cwv, cap):
        n = len(tl)
        tpad = np.zeros(cap, np.int64)
        tpad[:n] = tl
        cpad = np.zeros(((cap + P - 1) // P) * P, f32)
        cpad[:n] = cwv
        xg = np.ascontiguousarray(xTq[:, tpad])
        cwcol = np.ascontiguousarray(
            cpad.reshape(-1, P).T)
        return xg, cwcol

    sh_halves = halves(sw1, sb1, sw3, sb3, sw2, sb2)

    in_maps = [dict() for _ in range(NCORES)]
    meta = []
    for g in range(NGROUPS):
        eA, eB = eAs[g], eBs[g]
        xgA, cwA = gather(toks[eA], cws[eA], CAP_A)
        xgB, cwB = gather(toks[eB], cws[eB], CAP_B)
        ssl = slice(g * CAP_S, (g + 1) * CAP_S)
        xs = np.ascontiguousarray(xTq[:, ssl])
        scw = np.ones((P, CAP_S // P), f32)
        meta.append((toks[eA], counts[eA], np.asarray(cws[eA], f32), eA,
                     toks[eB], counts[eB], np.asarray(cws[eB], f32), eB))
        hvA = halves(w1[eA], b1[eA], w3[eA], b3[eA], w2[eA], b2[eA])
        hvB = halves(w1[eB], b1[eB], w3[eB], b3[eB], w2[eB], b2[eB])
        for h in range(2):
            c = 2 * g + h
            m = in_maps[c]
            for pref, hv in (("a_", hvA[h]), ("b_", hvB[h])):
                for k, v in hv.items():
                    m[pref + k] = v
            for k, v in sh_halves[h].items():
                m["s_" + k] = v
            m["a_xT"], m["a_cw"] = xgA, cwA
            m["b_xT"], m["b_cw"] = xgB, cwB
            m["s_xT"], m["s_cw"] = xs, scw
    return in_maps, meta, None


_PROGS = {}


def _get_program(kind):
    if kind not in _PROGS:
        _PROGS[kind] = {"sparse": _build_sparse, "dense": _build_dense}[kind]()
    return _PROGS[kind]


def kernel(x, gate_w, gate_b, w1, b1, w3, b3, w2, b2,
           sw1, sb1, sw3, sb3, sw2, sb2, _trace=False, _results=None,
           _force_dense=False):
    kw = {}
    if _trace:
        kw = dict(trace=True, trace_cores=list(range(NCORES)))
    args = (x, gate_w, gate_b, w1, b1, w3, b3, w2, b2,
            sw1, sb1, sw3, sb3, sw2, sb2)
    if not _force_dense:
        in_maps, meta, _ = _prep_sparse(*args)
    else:
        in_maps = None
    if in_maps is not None:
        nc = _get_program("sparse")
        res = run_bass_kernel_spmd(
            nc, in_maps, core_ids=list(range(NCORES)), **kw)
        if _results is not None:
            _results.append(res)
        f32 = np.float32
        out = np.zeros((T, D), f32)
        for g in range(NGROUPS):
            r0, r1 = res.results[2 * g], res.results[2 * g + 1]
            out[g * CAP_S:(g + 1) * CAP_S] = (
                r0["s_out"].astype(f32) + r1["s_out"].astype(f32))
        out += np.asarray(sb2, f32)          # shared-expert down bias
        for g in range(NGROUPS):
            r0, r1 = res.results[2 * g], res.results[2 * g + 1]
            tA, nA, cwA, eA, tB, nB, cwB, eB = meta[g]
            out[tA] += (r0["a_out"][:nA].astype(f32)
                        + r1["a_out"][:nA].astype(f32)
                        + cwA[:nA, None] * np.asarray(b2[eA], f32))
            out[tB] += (r0["b_out"][:nB].astype(f32)
                        + r1["b_out"][:nB].astype(f32)
                        + cwB[:nB, None] * np.asarray(b2[eB], f32))
        return out.reshape(np.asarray(x).shape).astype(np.float32)

    # dense fallback
    in_maps = _prep_dense(*args)
    nc = _get_program("dense")
    res = run_bass_kernel_spmd(nc, in_maps, core_ids=list(range(NCORES)), **kw)
    if _results is not None:
        _results.append(res)
    acc = np.zeros((T, D), np.float32)
    for c in range(NCORES):
        acc += res.results[c]["out"]
    return acc.reshape(np.asarray(x).shape).astype(np.float32)


# ---------------------------------------------------------------------------
# dense all-on-device fallback (V1): every core runs its expert over all
# tokens, masked by on-device gate weights; shared expert sharded on 2F.
# ---------------------------------------------------------------------------

TCH = 512
NTH = 2
TH = T // NTH
DKF = D // P
FIF = F // P


def _build_dense():
    nc = bacc.Bacc(
        "TRN2", target_bir_lowering=False, debug=False, num_devices=NCORES
    )
    aps = {}

    def inp(name, shape, dt=F32):
        aps[name] = nc.dram_tensor(name, shape, dt, kind="ExternalInput").ap()

    inp("xT", [D, T], F32R)
    inp("gw", [P, DKF * E], F32R)
    inp("gb", [P, E])
    inp("sel", [P, E])
    for n in ("w1g", "w1l", "w3g", "w3l"):
        inp(n, [FIF, P, DKF, P], F32R)
    for n in ("b1g", "b1l", "b3g", "b3l"):
        inp(n, [P, FIF + 1])
    inp("w2T", [F, D], F32R)
    inp("b2r", [1, D], F32R)
    for n in ("sw1g", "sw1l", "sw3g", "sw3l"):
        inp(n, [P, DKF, P], F32R)
    inp("sw2T", [P, D], F32R)
    inp("sb2r", [1, D], F32R)
    inp("ones", [1, P], F32R)
    aps["out"] = nc.dram_tensor("out", [T, D], F32, kind="ExternalOutput").ap()

    with tile.TileContext(nc) as tc:
        _emit_dense(tc, aps)
    nc.compile()
    return nc


def _emit_dense(tc, aps):
    nc = tc.nc
    ctx = ExitStack()

    with ctx:
        const = ctx.enter_context(tc.tile_pool(name="const", bufs=1))

        xsb = []
        for dk in range(DKF):
            t = const.tile([P, T], F32R, tag=f"x{dk}")
            nc.sync.dma_start(t[:], aps["xT"][dk * P:(dk + 1) * P, :])
            xsb.append(t)

        def load_const(name, shape, dt=F32):
            t = const.tile(shape, dt, tag=name)
            nc.sync.dma_start(t[:], aps[name][:])
            return t

        gw_sb = load_const("gw", [P, DKF * E], F32R)
        gb_sb = load_const("gb", [P, E])
        sel_sb = load_const("sel", [P, E])
        bcols = {n: load_const(n, [P, FIF + 1])
                 for n in ("b1g", "b1l", "b3g", "b3l")}
        b2r_sb = load_const("b2r", [1, D], F32R)
        sb2r_sb = load_const("sb2r", [1, D], F32R)
        sw2T_sb = load_const("sw2T", [P, D], F32R)
        ssw = {}
        for name in ("sw1g", "sw1l", "sw3g", "sw3l"):
            t = const.tile([P, DKF, P], F32R, tag=name)
            nc.sync.dma_start(t[:], aps[name][:])
            ssw[name] = t

        ones = const.tile([1, P], F32R, tag="ones")
        nc.sync.dma_start(ones[:], aps["ones"][:])
        ident = const.tile([E, E], F32, tag="ident")
        nc.vector.memset(ident[:], 0.0)
        from concourse.masks import make_identity
        make_identity(nc, ident[:], nomemset=True)

        cw = const.tile([P, T // P], F32, tag="cw")

        # ---- gate ----
        with tc.tile_pool(name="psG", bufs=2, space="PSUM") as psG, \
             tc.tile_pool(name="gtmp", bufs=1) as gtmp:
            NC = T // P
            logits_tb = const.tile([P, NC * E], F32, tag="logits_tb")
            logitsT = gtmp.tile([E, T], F32, tag="logitsT")
            for tch in range(T // TCH):
                pg = psG.tile([E, TCH], F32, tag="pslog")
                for dk in range(DKF):
                    nc.tensor.matmul(
                        pg[:],
                        (gw_sb[:, dk * E:(dk + 1) * E]),
                        (xsb[dk][:, tch * TCH:(tch + 1) * TCH]),
                        start=(dk == 0), stop=(dk == DKF - 1),
                    )
                nc.scalar.copy(logitsT[:, tch * TCH:(tch + 1) * TCH], pg[:])
            for j in range(NC):
                pt = psG.tile([P, E], F32, tag="pstr")
                nc.tensor.transpose(
                    pt[:], logitsT[:, j * P:(j + 1) * P], ident[:])
                nc.scalar.copy(logits_tb[:, j * E:(j + 1) * E], pt[:])

            eL = gtmp.tile([P, NC * E], F32, tag="eL")
            nc.scalar.activation(eL[:], logits_tb[:], AF.Exp)
            e3 = eL[:].rearrange("p (c e) -> p c e", e=E)
            ssum = gtmp.tile([P, NC], F32, tag="ssum")
            nc.vector.reduce_sum(ssum[:], e3, axis=mybir.AxisListType.X)
            rs = gtmp.tile([P, NC], F32, tag="rs")
            nc.vector.reciprocal(rs[:], ssum[:])
            scores = gtmp.tile([P, NC * E], F32, tag="scores")
            s3 = scores[:].rearrange("p (c e) -> p c e", e=E)
            nc.vector.tensor_mul(
                s3, e3, rs[:, :, None].broadcast_to((P, NC, E)))
            biased = gtmp.tile([P, NC * E], F32, tag="biased")
            bi3 = biased[:].rearrange("p (c e) -> p c e", e=E)
            nc.vector.tensor_add(
                bi3, s3, gb_sb[:, None, :].broadcast_to((P, NC, E)))
            m1 = gtmp.tile([P, NC], F32, tag="m1")
            nc.vector.reduce_max(m1[:], bi3, axis=mybir.AxisListType.X)
            mask1 = gtmp.tile([P, NC * E], F32, tag="mask1")
            mk3 = mask1[:].rearrange("p (c e) -> p c e", e=E)
            nc.vector.tensor_tensor(
                mk3, bi3, m1[:, :, None].broadcast_to((P, NC, E)), OP.is_ge)
            biased2 = gtmp.tile([P, NC * E], F32, tag="biased2")
            b23 = biased2[:].rearrange("p (c e) -> p c e", e=E)
            nc.vector.scalar_tensor_tensor(
                out=b23, in0=mk3, scalar=-1e30, in1=bi3,
                op0=OP.mult, op1=OP.add)
            m2 = gtmp.tile([P, NC], F32, tag="m2")
            nc.vector.reduce_max(m2[:], b23, axis=mybir.AxisListType.X)
            mask2 = gtmp.tile([P, NC * E], F32, tag="mask2")
            mq3 = mask2[:].rearrange("p (c e) -> p c e", e=E)
            nc.vector.tensor_tensor(
                mq3, bi3, m2[:, :, None].broadcast_to((P, NC, E)), OP.is_ge)
            cwf = gtmp.tile([P, NC * E], F32, tag="cwf")
            cf3 = cwf[:].rearrange("p (c e) -> p c e", e=E)
            nc.vector.tensor_mul(cf3, s3, mq3)
            nc.vector.tensor_mul(
                cf3, cf3, sel_sb[:, None, :].broadcast_to((P, NC, E)))
            nc.vector.reduce_sum(cw[:], cf3, axis=mybir.AxisListType.X)

        # ---- main ----
        wA = ctx.enter_context(tc.tile_pool(name="wA", bufs=2))
        w2p = ctx.enter_context(tc.tile_pool(name="w2p", bufs=3))
        apool = ctx.enter_context(tc.tile_pool(name="apool", bufs=1))
        hpool = ctx.enter_context(tc.tile_pool(name="hpool", bufs=2))
        outp = ctx.enter_context(tc.tile_pool(name="outp", bufs=3))
        psA = ctx.enter_context(tc.tile_pool(name="psA", bufs=2, space="PSUM"))
        psB = ctx.enter_context(tc.tile_pool(name="psB", bufs=2, space="PSUM"))
        psS = ctx.enter_context(tc.tile_pool(name="psS", bufs=2, space="PSUM"))

        afc = FIF + 1
        for th in range(NTH):
            tbase = th * TH
            atiles = []
            for fi in range(afc):
                at = apool.tile([P, TH], F32R, tag=f"a{fi}")
                atiles.append(at)
                if fi < FIF:
                    wt = {}
                    for nm in ("w1g", "w1l", "w3g", "w3l"):
                        t = wA.tile([P, DKF, P], F32R, tag=nm)
                        nc.sync.dma_start(t[:], aps[nm][fi])
                        wt[nm] = t
                    w_g1, w_l1 = wt["w1g"], wt["w1l"]
                    w_g3, w_l3 = wt["w3g"], wt["w3l"]
                else:
                    w_g1, w_l1 = ssw["sw1g"], ssw["sw1l"]
                    w_g3, w_l3 = ssw["sw3g"], ssw["sw3l"]
                bc1g = bcols["b1g"][:, fi:fi + 1]
                bc1l = bcols["b1l"][:, fi:fi + 1]
                bc3g = bcols["b3g"][:, fi:fi + 1]
                bc3l = bcols["b3l"][:, fi:fi + 1]

                for tt in range(TH // TCH):
                    tsl = slice(tt * TCH, (tt + 1) * TCH)
                    gsl = slice(tbase + tt * TCH, tbase + (tt + 1) * TCH)

                    def hpsum(wtile, ptag):
                        ps = psA.tile([P, TCH], F32, tag=ptag)
                        for dk in range(DKF):
                            nc.tensor.matmul(
                                ps[:], (wtile[:, dk, :]),
                                (xsb[dk][:, gsl]),
                                start=(dk == 0), stop=(dk == DKF - 1))
                        return ps

                    pg1 = hpsum(w_g1, "pA")
                    t1 = hpool.tile([P, TCH], F32, tag="tcp")
                    nc.scalar.activation(t1[:], pg1[:], AF.Identity, bias=bc1g)
                    pg3 = hpsum(w_g3, "pB")
                    hg = hpool.tile([P, TCH], F32, tag="hh")
                    nc.vector.scalar_tensor_tensor(
                        out=hg[:], in0=pg3[:], scalar=bc3g, in1=t1[:],
                        op0=OP.add, op1=OP.mult)
                    nc.vector.tensor_scalar_min(hg[:], hg[:], LIMIT)
                    gs = hpool.tile([P, TCH], F32, tag="gs")
                    nc.scalar.activation(gs[:], hg[:], AF.Silu, scale=ALPHA)

                    pl1 = hpsum(w_l1, "pA")
                    t2 = hpool.tile([P, TCH], F32, tag="tcp")
                    nc.scalar.activation(t2[:], pl1[:], AF.Identity, bias=bc1l)
                    pl3 = hpsum(w_l3, "pB")
                    hl = hpool.tile([P, TCH], F32, tag="hh")
                    nc.vector.scalar_tensor_tensor(
                        out=hl[:], in0=pl3[:], scalar=bc3l, in1=t2[:],
                        op0=OP.add, op1=OP.mult)
                    nc.vector.tensor_scalar(
                        out=hl[:], in0=hl[:], scalar1=LIMIT, scalar2=-LIMIT,
                        op0=OP.min, op1=OP.max)
                    nc.vector.tensor_scalar(
                        out=hl[:], in0=hl[:], scalar1=1.0 / ALPHA,
                        scalar2=1.0 / ALPHA, op0=OP.mult, op1=OP.add)
                    nc.vector.tensor_mul(atiles[fi][:, tsl], gs[:], hl[:])

            for tp in range(TH // P):
                j = th * (TH // P) + tp
                tsl = slice(tp * P, (tp + 1) * P)
                for dch in range(D // TCH):
                    dsl = slice(dch * TCH, (dch + 1) * TCH)
                    pB = psB.tile([P, TCH], F32, tag="pB2")
                    nc.tensor.matmul(pB[:], (ones[:]),
                                     (b2r_sb[0:1, dsl]),
                                     start=True, stop=False)
                    for fi in range(FIF):
                        wt2 = w2p.tile([P, TCH], F32R, tag="w2t")
                        nc.sync.dma_start(
                            wt2[:], aps["w2T"][fi * P:(fi + 1) * P, dsl])
                        nc.tensor.matmul(
                            pB[:], (atiles[fi][:, tsl]), (wt2[:]),
                            start=False, stop=(fi == FIF - 1))
                    pS = psS.tile([P, TCH], F32, tag="pS")
                    nc.tensor.matmul(pS[:], (ones[:]),
                                     (sb2r_sb[0:1, dsl]),
                                     start=True, stop=False)
                    nc.tensor.matmul(
                        pS[:], (atiles[FIF][:, tsl]), (sw2T_sb[:, dsl]),
                        start=False, stop=True)
                    ot = outp.tile([P, TCH], F32, tag="ot")
                    nc.vector.tensor_scalar_mul(ot[:], pB[:], cw[:, j:j + 1])
                    nc.vector.tensor_add(ot[:], pS[:], ot[:])
                    nc.sync.dma_start(
                        aps["out"][tbase + tp * P:tbase + (tp + 1) * P, dsl],
                        ot[:])


def _prep_dense(x, gate_w, gate_b, w1, b1, w3, b3, w2, b2,
                sw1, sb1, sw3, sb3, sw2, sb2):
    f32 = np.float32
    xt = np.asarray(x, f32).reshape(T, D)
    xT = np.ascontiguousarray(xt.T)
    gwT = np.asarray(gate_w, f32).T
    gw_sb = np.ascontiguousarray(
        gwT.reshape(DKF, P, E).transpose(1, 0, 2).reshape(P, DKF * E))
    gb_bc = np.ascontiguousarray(
        np.broadcast_to(np.asarray(gate_b, f32), (P, E)))

    sw1 = np.asarray(sw1, f32)
    sw3 = np.asarray(sw3, f32)
    sw2T = np.asarray(sw2, f32).T
    sb1 = np.asarray(sb1, f32)
    sb3 = np.asarray(sb3, f32)
    sb2 = np.asarray(sb2, f32)

    def fwarr(w):      # [F, D] -> [FIF, P, DKF, P]
        return np.ascontiguousarray(
            w.T.reshape(DKF, P, FIF, P).transpose(2, 1, 0, 3))

    def swarr(w_sl):
        return np.ascontiguousarray(
            w_sl.T.reshape(DKF, P, P).transpose(1, 0, 2))

    def bcol2(b, sb_sl):
        return np.ascontiguousarray(
            np.concatenate([b.reshape(FIF, P).T, sb_sl[:, None]], axis=1))

    in_maps = []
    for c in range(NCORES):
        sel = np.zeros((P, E), f32)
        sel[:, c] = 1.0
        w1c = np.asarray(w1[c], f32)
        w3c = np.asarray(w3[c], f32)
        b1c = np.asarray(b1[c], f32)
        b3c = np.asarray(b3[c], f32)
        fsl = slice(c * P, (c + 1) * P)
        m = {
            "xT": xT, "gw": gw_sb, "gb": gb_bc, "sel": sel,
            "w1g": fwarr(w1c[0::2]), "w1l": fwarr(w1c[1::2]),
            "w3g": fwarr(w3c[0::2]), "w3l": fwarr(w3c[1::2]),
            "b1g": bcol2(b1c[0::2], sb1[0::2][fsl]),
            "b1l": bcol2(b1c[1::2], sb1[1::2][fsl]),
            "b3g": bcol2(b3c[0::2], sb3[0::2][fsl]),
            "b3l": bcol2(b3c[1::2], sb3[1::2][fsl]),
            "w2T": np.ascontiguousarray(np.asarray(w2[c], f32).T),
            "b2r": np.asarray(b2[c], f32)[None, :],
            "sw1g": swarr(sw1[0::2][fsl]), "sw1l": swarr(sw1[1::2][fsl]),
            "sw3g": swarr(sw3[0::2][fsl]), "sw3l": swarr(sw3[1::2][fsl]),
            "sw2T": np.ascontiguousarray(sw2T[fsl]),
            "sb2r": (sb2 if c == 0 else np.zeros_like(sb2))[None, :],
            "ones": np.ones((1, P), f32),
        }
        in_maps.append(m)
    return in_maps


if __name__ == "__main__":
    rng = np.random.RandomState(0)
    sd = 1 / 32.0
    ins = {
        "x": rng.randn(2, 1024, 1024).astype(np.float32),
        "gate_w": (rng.randn(E, D) * sd).astype(np.float32),
        "gate_b": (rng.randn(E) * 0.01).astype(np.float32),
        "w1": (rng.randn(E, 2 * F, D) * sd).astype(np.float32),
        "b1": (rng.randn(E, 2 * F) * 0.01).astype(np.float32),
        "w3": (rng.randn(E, 2 * F, D) * sd).astype(np.float32),
        "b3": (rng.randn(E, 2 * F) * 0.01).astype(np.float32),
        "w2": (rng.randn(E, D, F) * sd).astype(np.float32),
        "b2": (rng.randn(E, D) * 0.01).astype(np.float32),
        "sw1": (rng.randn(2 * F, D) * sd).astype(np.float32),
        "sb1": (rng.randn(2 * F) * 0.01).astype(np.float32),
        "sw3": (rng.randn(2 * F, D) * sd).astype(np.float32),
        "sb3": (rng.randn(2 * F) * 0.01).astype(np.float32),
        "sw2": (rng.randn(D, F) * sd).astype(np.float32),
        "sb2": (rng.randn(D) * 0.01).astype(np.float32),
    }
    out = kernel(**ins)
    print("OK", out.shape, out.dtype, np.abs(out).mean())
